# revision 4
# baseline (speedup 1.0000x reference)
"""Trainium2 Bass/Tile MoE kernel (top-2 of 16 experts, gelu MLP + per-expert LN).

Strategy: data-parallel over the batch dim (8 batch rows -> 8 NeuronCores),
experts replicated. Each core, fully on-device:
  1. gate scores in fp32 (PE transpose + matmul), top-2 via DVE Max/MaxIndex,
     softmax weights via ACT sigmoid
  2. routing tables built with a matmul prefix-sum over one-hots and
     indirect-DMA scatters of (token_id, weight) records into a DRAM table
  3. per-expert token dispatch via the gpsimd dma_gather custom instruction
     (HBM row gather + transpose to feature-major, bf16)
  4. expert MLP in bf16 on PE, gelu + bias on ACT, layernorm on DVE/ACT
  5. combine via dma_scatter_add (scaled rows accumulated at token positions)
Host only slices the batch, casts weight copies to bf16, and stacks outputs.
"""

import sys

for _p in ("/opt/trn_rl_repo",):
    if _p not in sys.path:
        sys.path.insert(0, _p)

import numpy as np
import ml_dtypes

import concourse.bass as bass
import concourse.mybir as mybir
import concourse.tile as tile
from concourse import bacc, library_config
from concourse.bass import IndirectOffsetOnAxis
from concourse.bass_utils import run_bass_kernel_spmd
from concourse.masks import make_identity, make_upper_triangular

P = 128
B, S, D, F, E = 8, 4096, 512, 1024, 16
N = S                 # tokens per core (one batch row per core)
NT = N // P           # token tiles per core
CAP = 640             # per-expert slot capacity (multiple of 128)
NCAP = CAP // P
DUMP = 128            # overflow dump rows at the end of the routing table
TROWS = E * CAP + DUMP
LN_EPS = 1e-5

FP32 = mybir.dt.float32
BF16 = mybir.dt.bfloat16
I16 = mybir.dt.int16
I32 = mybir.dt.int32
U32 = mybir.dt.uint32
AF = mybir.ActivationFunctionType
ALU = mybir.AluOpType
AXX = mybir.AxisListType.X


def _gate_tile(nc, tc, ti, pools, aps):
    """Gate + routing-record scatter for one 128-token tile."""
    const, gp, gps = pools["const"], pools["gate"], pools["gpsum"]
    c = pools["cvals"]
    x32, table = aps["x32"], aps["table"]

    xt = gp.tile([P, D], FP32, tag="xt")
    nc.sync.dma_start(xt, x32[ti * P:(ti + 1) * P, :])
    xT = gp.tile([P, D // P, P], FP32, tag="xT")
    for g in range(D // P):
        pt = gps.tile([P, P], FP32, tag="g")
        nc.tensor.transpose(pt, xt[:, g * P:(g + 1) * P], c["ident"])
        nc.scalar.copy(xT[:, g, :], pt)

    scps = gps.tile([P, E], FP32, tag="g")
    for g in range(D // P):
        nc.tensor.matmul(scps, lhsT=xT[:, g, :], rhs=c["gw_sb"][:, g, :],
                         start=(g == 0), stop=False)
    nc.tensor.matmul(scps, lhsT=c["ones_row"], rhs=c["gb_sb"],
                     start=False, stop=True)
    sc = gp.tile([P, E], FP32, tag="sc")
    nc.vector.tensor_copy(sc, scps)

    mx8 = gp.tile([P, 8], FP32, tag="mx8")
    nc.vector.max(mx8, sc)
    ix8 = gp.tile([P, 8], U32, tag="ix8")
    nc.vector.max_index(ix8, mx8, sc)

    d12 = gp.tile([P, 1], FP32, tag="d12")
    nc.vector.tensor_tensor(d12, mx8[:, 0:1], mx8[:, 1:2], op=ALU.subtract)
    wk0 = gp.tile([P, 1], FP32, tag="wk0")
    nc.scalar.activation(wk0, d12, AF.Sigmoid)
    wk1 = gp.tile([P, 1], FP32, tag="wk1")
    nc.vector.tensor_tensor(wk1, c["ones_col"], wk0, op=ALU.subtract)

    ef = gp.tile([P, 2], FP32, tag="ef")
    nc.vector.tensor_copy(ef, ix8[:, 0:2])
    A0 = gp.tile([P, E], FP32, tag="A0")
    nc.vector.tensor_tensor(A0, ef[:, 0:1].to_broadcast([P, E]), c["iota_e"],
                            op=ALU.is_equal)
    A1 = gp.tile([P, E], FP32, tag="A1")
    nc.vector.tensor_tensor(A1, ef[:, 1:2].to_broadcast([P, E]), c["iota_e"],
                            op=ALU.is_equal)
    A = gp.tile([P, E], FP32, tag="A")
    nc.vector.tensor_tensor(A, A0, A1, op=ALU.add)

    # base[e] = e*CAP + (tokens already routed to e in earlier tiles)
    base = gp.tile([1, E], FP32, tag="base")
    nc.vector.tensor_tensor(base, c["ecap"], c["running"], op=ALU.add)
    pps = gps.tile([P, E], FP32, tag="g")
    nc.tensor.matmul(pps, lhsT=c["ustrict"], rhs=A, start=True, stop=False)
    nc.tensor.matmul(pps, lhsT=c["ones_row"], rhs=base, start=False, stop=True)
    dall = gp.tile([P, E], FP32, tag="dall")
    nc.vector.tensor_copy(dall, pps)
    cps = gps.tile([1, E], FP32, tag="g")
    nc.tensor.matmul(cps, lhsT=c["ones_col"], rhs=A, start=True, stop=True)
    nc.vector.tensor_tensor(c["running"], c["running"], cps, op=ALU.add)

    for k in range(2):
        Ak = A0 if k == 0 else A1
        wk = wk0 if k == 0 else wk1
        sel = gp.tile([P, E], FP32, tag=f"sel{k}")
        nc.vector.tensor_tensor(sel, Ak, dall, op=ALU.mult)
        dk = gp.tile([P, 1], FP32, tag=f"dk{k}")
        nc.vector.tensor_reduce(dk, sel, axis=AXX, op=ALU.add)
        nc.vector.tensor_scalar_min(dk, dk, float(TROWS - 1))
        dki = gp.tile([P, 1], I32, tag=f"dki{k}")
        nc.vector.tensor_copy(dki, dk)
        rec = gp.tile([P, 2], FP32, tag=f"rec{k}")
        nc.vector.tensor_scalar_add(rec[:, 0:1], c["iota_p"], float(ti * P))
        nc.vector.tensor_copy(rec[:, 1:2], wk)
        nc.gpsimd.indirect_dma_start(
            out=table[:, :],
            out_offset=IndirectOffsetOnAxis(ap=dki[:, :1], axis=0),
            in_=rec[:, :],
            in_offset=None,
        )


def _expert(nc, tc, e, pools, aps):
    """Dispatch + MLP + LN + combine for one expert."""
    const, ep, eps = pools["const"], pools["ep"], pools["epsum"]
    c = pools["cvals"]
    xbf, table, out = aps["xbf"], aps["table"], aps["out"]
    w1, b1, w2, b2 = aps["w1"], aps["b1"], aps["w2"], aps["b2"]
    gamma, beta = aps["gamma"], aps["beta"]

    # token-index list, wrapped [j%16, j//16] and replicated to the 8 Q7 cores
    idxf = ep.tile([P, CAP // 16], FP32, tag="idxf")
    col0 = table[e * CAP:(e + 1) * CAP, 0:1].rearrange(
        "(w p) one -> p (w one)", p=16)
    for q in range(8):
        nc.sync.dma_start(idxf[16 * q:16 * (q + 1), :], col0)
    idx16 = ep.tile([P, CAP // 16], I16, tag="idx16")
    nc.vector.tensor_copy(idx16, idxf)
    # per-slot gate weight, [i%128, i//128]
    wcol = ep.tile([P, NCAP], FP32, tag="wcol")
    nc.sync.dma_start(wcol, table[e * CAP:(e + 1) * CAP, 1:2].rearrange(
        "(g p) one -> p (g one)", p=P))

    xdT = ep.tile([P, D // P, CAP], BF16, tag="xdT")
    nc.gpsimd.dma_gather(xdT[:], xbf[:, :], idx16[:], CAP, CAP, D,
                         transpose=True)

    w1sb = ep.tile([P, D // P, F], BF16, tag="w1sb")
    nc.sync.dma_start(w1sb, w1[e].rearrange("(g p) f -> p g f", p=P))
    w2sb = ep.tile([P, F // P, D], BF16, tag="w2sb")
    nc.sync.dma_start(w2sb, w2[e].rearrange("(k p) d -> p k d", p=P))
    b1sb = ep.tile([P, F // P], FP32, tag="b1sb")
    nc.sync.dma_start(b1sb, b1[e].rearrange("(f p) -> p f", p=P))
    b2row = ep.tile([1, D], BF16, tag="b2row")
    nc.sync.dma_start(b2row, b2[e][None, :])

    # broadcast gamma/beta rows across partitions via K=1 matmul
    grow = ep.tile([1, D], FP32, tag="grow")
    nc.sync.dma_start(grow, gamma[e][None, :])
    brow = ep.tile([1, D], FP32, tag="brow")
    nc.sync.dma_start(brow, beta[e][None, :])
    gpsb = eps.tile([P, D], FP32, tag="e")
    nc.tensor.matmul(gpsb, lhsT=c["ones_row"], rhs=grow, start=True, stop=True)
    gam_b = ep.tile([P, D], FP32, tag="gam_b")
    nc.scalar.copy(gam_b, gpsb)
    bpsb = eps.tile([P, D], FP32, tag="e")
    nc.tensor.matmul(bpsb, lhsT=c["ones_row"], rhs=brow, start=True, stop=True)
    bet_b = ep.tile([P, D], FP32, tag="bet_b")
    nc.scalar.copy(bet_b, bpsb)

    # hT[f, slot] = gelu(x @ w1 + b1)^T, built 128 features at a time
    hT = ep.tile([P, F // P, CAP], BF16, tag="hT")
    for ft in range(F // P):
        for c0, cw in ((0, 512), (512, CAP - 512)):
            hps = eps.tile([P, 512], FP32, tag="e")
            for g in range(D // P):
                nc.tensor.matmul(hps[:, :cw],
                                 lhsT=w1sb[:, g, ft * P:(ft + 1) * P],
                                 rhs=xdT[:, g, c0:c0 + cw],
                                 start=(g == 0), stop=(g == D // P - 1))
            nc.scalar.activation(hT[:, ft, c0:c0 + cw], hps[:, :cw], AF.Gelu,
                                 bias=b1sb[:, ft:ft + 1], scale=1.0)

    yall = ep.tile([P, NCAP, D], FP32, tag="yall")
    for j in range(NCAP):
        yps = eps.tile([P, D], FP32, tag="e")
        for ks in range(F // P):
            nc.tensor.matmul(yps, lhsT=hT[:, ks, j * P:(j + 1) * P],
                             rhs=w2sb[:, ks, :],
                             start=(ks == 0), stop=False)
        nc.tensor.matmul(yps, lhsT=c["ones_row_bf"], rhs=b2row,
                         start=False, stop=True)

        # layernorm over D (free dim), then *gamma +beta, then * gate weight
        ysc = ep.tile([P, D], FP32, tag="ysc")
        sumy = ep.tile([P, 1], FP32, tag="sumy")
        nc.scalar.activation(ysc, yps, AF.Copy, accum_out=sumy)
        sq = ep.tile([P, D], FP32, tag="sq")
        sumsq = ep.tile([P, 1], FP32, tag="sumsq")
        nc.scalar.activation(sq, yps, AF.Square, accum_out=sumsq)
        mu = ep.tile([P, 1], FP32, tag="mu")
        nc.vector.tensor_scalar_mul(mu, sumy, 1.0 / D)
        musq = ep.tile([P, 1], FP32, tag="musq")
        nc.vector.tensor_tensor(musq, mu, mu, op=ALU.mult)
        var = ep.tile([P, 1], FP32, tag="var")
        nc.vector.tensor_scalar(var, sumsq, 1.0 / D, None, op0=ALU.mult)
        nc.vector.tensor_tensor(var, var, musq, op=ALU.subtract)
        sd = ep.tile([P, 1], FP32, tag="sd")
        nc.scalar.activation(sd, var, AF.Sqrt, bias=c["eps_col"][:, :1])
        rstd = ep.tile([P, 1], FP32, tag="rstd")
        nc.vector.reciprocal(rstd, sd)
        ap_ = ep.tile([P, 1], FP32, tag="ap_")
        nc.vector.tensor_tensor(ap_, rstd, wcol[:, j:j + 1], op=ALU.mult)
        z = ep.tile([P, D], FP32, tag="z")
        nc.vector.tensor_scalar(z, ysc, mu, ap_, op0=ALU.subtract, op1=ALU.mult)
        bw = ep.tile([P, D], FP32, tag="bw")
        nc.scalar.activation(bw, bet_b, AF.Copy, scale=wcol[:, j:j + 1])
        nc.vector.tensor_tensor(yall[:, j, :], z, gam_b, op=ALU.mult)
        nc.vector.tensor_tensor(yall[:, j, :], yall[:, j, :], bw, op=ALU.add)

    nc.gpsimd.dma_scatter_add(out[:, :], yall[:], idx16[:], CAP, CAP, D)


def _moe_kernel(tc, aps):
    nc = tc.nc
    from contextlib import ExitStack
    with ExitStack() as st:
        const = st.enter_context(tc.tile_pool(name="const", bufs=1))
        gp = st.enter_context(tc.tile_pool(name="gate", bufs=3))
        gps = st.enter_context(tc.tile_pool(name="gpsum", bufs=4, space="PSUM"))
        ep = st.enter_context(tc.tile_pool(name="ep", bufs=2))
        eps = st.enter_context(tc.tile_pool(name="epsum", bufs=4, space="PSUM"))

        nc.gpsimd.load_library(library_config.mlp)

        cv = {}
        cv["ident"] = const.tile([P, P], FP32, name="ident")
        make_identity(nc, cv["ident"])
        cv["ustrict"] = const.tile([P, P], FP32, name="ustrict")
        make_upper_triangular(nc, cv["ustrict"], val=1.0, diag=False)
        cv["ones_row"] = const.tile([1, P], FP32, name="ones_row")
        nc.vector.memset(cv["ones_row"], 1.0)
        cv["ones_row_bf"] = const.tile([1, P], BF16, name="ones_row_bf")
        nc.vector.memset(cv["ones_row_bf"], 1.0)
        cv["ones_col"] = const.tile([P, 1], FP32, name="ones_col")
        nc.vector.memset(cv["ones_col"], 1.0)
        iota_e_i = const.tile([P, E], I32)
        nc.gpsimd.iota(iota_e_i, pattern=[[1, E]], base=0, channel_multiplier=0)
        cv["iota_e"] = const.tile([P, E], FP32, name="iota_e")
        nc.vector.tensor_copy(cv["iota_e"], iota_e_i)
        iota_p_i = const.tile([P, 1], I32)
        nc.gpsimd.iota(iota_p_i, pattern=[[0, 1]], base=0, channel_multiplier=1)
        cv["iota_p"] = const.tile([P, 1], FP32, name="iota_p")
        nc.vector.tensor_copy(cv["iota_p"], iota_p_i)
        ecap_i = const.tile([1, E], I32)
        nc.gpsimd.iota(ecap_i, pattern=[[1, E]], base=0, channel_multiplier=0)
        cv["ecap"] = const.tile([1, E], FP32, name="ecap")
        nc.vector.tensor_copy(cv["ecap"], ecap_i)
        nc.vector.tensor_scalar_mul(cv["ecap"], cv["ecap"], float(CAP))
        cv["gw_sb"] = const.tile([P, D // P, E], FP32, name="gw_sb")
        nc.sync.dma_start(cv["gw_sb"], aps["gw"].rearrange("(g p) e -> p g e", p=P))
        cv["gb_sb"] = const.tile([1, E], FP32, name="gb_sb")
        nc.sync.dma_start(cv["gb_sb"], aps["gb"][None, :])
        cv["running"] = const.tile([1, E], FP32, name="running")
        nc.vector.memset(cv["running"], 0.0)
        zero_col = const.tile([P, 1], FP32, name="zero_col")
        nc.vector.memset(zero_col, 0.0)
        eps_col = const.tile([P, 1], FP32, name="eps_col")
        nc.vector.memset(eps_col, LN_EPS)
        nc.const_aps.aps[(FP32, 0.0)] = zero_col[:]
        cv["eps_col"] = eps_col

        # zero-init the routing table (pad slots read token 0 with weight 0)
        zrow = const.tile([P, TROWS * 2 // P], FP32)
        nc.vector.memset(zrow, 0.0)
        nc.sync.dma_start(
            aps["table"].rearrange("(p c) two -> p (c two)", p=P), zrow)

        pools = {"const": const, "gate": gp, "gpsum": gps, "ep": ep,
                 "epsum": eps, "cvals": cv}

        for ti in range(NT):
            _gate_tile(nc, tc, ti, pools, aps)
        for e in range(E):
            _expert(nc, tc, e, pools, aps)


_NC_CACHE = None


def _build_nc():
    nc = bacc.Bacc("TRN2", target_bir_lowering=False, debug=False,
                   num_devices=8)
    t = {}
    t["x32"] = nc.dram_tensor("x32", [N, D], FP32, kind="ExternalInput")
    t["xbf"] = nc.dram_tensor("xbf", [N, D], BF16, kind="ExternalInput")
    t["gw"] = nc.dram_tensor("gw", [D, E], FP32, kind="ExternalInput")
    t["gb"] = nc.dram_tensor("gb", [E], FP32, kind="ExternalInput")
    t["w1"] = nc.dram_tensor("w1", [E, D, F], BF16, kind="ExternalInput")
    t["b1"] = nc.dram_tensor("b1", [E, F], FP32, kind="ExternalInput")
    t["w2"] = nc.dram_tensor("w2", [E, F, D], BF16, kind="ExternalInput")
    t["b2"] = nc.dram_tensor("b2", [E, D], BF16, kind="ExternalInput")
    t["gamma"] = nc.dram_tensor("gamma", [E, D], FP32, kind="ExternalInput")
    t["beta"] = nc.dram_tensor("beta", [E, D], FP32, kind="ExternalInput")
    t["out"] = nc.dram_tensor("out", [N, D], FP32, kind="ExternalOutput")
    t["table"] = nc.dram_tensor("table", [TROWS, 2], FP32)
    aps = {k: v.ap() for k, v in t.items()}
    with tile.TileContext(nc) as tc:
        _moe_kernel(tc, aps)
    nc.compile()
    return nc


def get_nc():
    global _NC_CACHE
    if _NC_CACHE is None:
        _NC_CACHE = _build_nc()
    return _NC_CACHE


LAST_RESULT = None


def make_in_maps(inputs):
    x = np.asarray(inputs["x"], np.float32)
    gw = np.ascontiguousarray(np.asarray(inputs["gate_w"], np.float32))
    gb = np.ascontiguousarray(np.asarray(inputs["gate_b"], np.float32))
    w1 = np.asarray(inputs["w1"], np.float32)
    b1 = np.ascontiguousarray(np.asarray(inputs["b1"], np.float32))
    w2 = np.asarray(inputs["w2"], np.float32)
    b2 = np.asarray(inputs["b2"], np.float32)
    gamma = np.ascontiguousarray(np.asarray(inputs["gamma"], np.float32))
    beta = np.ascontiguousarray(np.asarray(inputs["beta"], np.float32))
    w1bf = np.ascontiguousarray(w1.astype(ml_dtypes.bfloat16))
    w2bf = np.ascontiguousarray(w2.astype(ml_dtypes.bfloat16))
    b2bf = np.ascontiguousarray(b2.astype(ml_dtypes.bfloat16))
    in_maps = []
    for i in range(B):
        xi = np.ascontiguousarray(x[i])
        in_maps.append({
            "x32": xi,
            "xbf": np.ascontiguousarray(xi.astype(ml_dtypes.bfloat16)),
            "gw": gw, "gb": gb, "w1": w1bf, "b1": b1, "w2": w2bf,
            "b2": b2bf, "gamma": gamma, "beta": beta,
        })
    return in_maps


def kernel(**inputs):
    global LAST_RESULT
    nc = get_nc()
    in_maps = make_in_maps(inputs)
    res = run_bass_kernel_spmd(nc, in_maps, list(range(B)))
    LAST_RESULT = res
    out = np.stack([np.asarray(r["out"], np.float32) for r in res.results],
                   axis=0)
    return np.ascontiguousarray(out).reshape(B, S, D)
